# revision 68
# baseline (speedup 1.0000x reference)
"""Self-contained Trainium2 kernel for nn_Attention_22814866276679.

Multi-head attention (ViT-style, 197 tokens, 12 heads, dim 768) with a
relative-position bias table, batch 64. Data-parallel over batch across the
8 NeuronCores (8 images per core, no collectives).

Math notes (host prep moves all layout work off the device):
  - qkv = x @ w_qkv.T + concat(q_bias, 0, v_bias); q *= 1/8. The 1/8 scale is
    folded into the pre-transposed q weights; the q bias is added per
    partition during the PSUM->SBUF copy (DVE tensor_scalar add); the v
    bias commutes past the softmax (weights sum to 1) so its projected image
    joins the output-projection bias.
  - scores are computed TRANSPOSED ([keys, queries]) so the softmax reduce
    (over keys) lands on the matmul contraction axis; no PE transposes.
  - relative-position bias: exp(s+b) = exp(s) * exp(b). The exp(bias) tables
    are precomputed on host (exact, bf16) and multiplied into the exp'd
    scores on DVE (one op per head-PAIR), replacing the per-head bias
    identity-matmuls on the PE. |scores| <= ~3 for these inputs, so exp()
    needs no max-subtraction (mathematically identical softmax).
  - V carries an appended ones column: the attention@V matmul then emits the
    softmax denominators as a 65th output row for free. The two heads of a
    pair share one PSUM bank, so one copy serves both; the denominators are
    broadcast to 64 partitions on GpSimd and the reciprocal runs on DVE.
    Normalization is deferred by two pairs so no engine queue head-of-line
    blocks on it.
  - The output projection is computed TRANSPOSED (outT = wp.T @ aoT,
    [dim, tokens]): the projection bias is then per-partition, so the
    PSUM->SBUF copy runs on the Act engine with a fused bias add, and the
    free dim (1576 tokens = 4*394) tiles evenly. The host transposes back.

Scheduling: the attention softmax chain has ~1.5us of cross-engine latency
per head-pair. To keep the PE busy through it, most non-attention matmul
work is DEFERRED and fed between pairs as filler: v-projection tiles past
the dense prefix, q/k projection chunks n>=1, and transposed-output-
projection tiles as their image pairs complete. Fillers pop at two points
inside each pair: unlocked projection tiles first, then an earliest-
deadline-first queue of the deferred v/qk work.
"""

import os
import sys

for _p in ("/opt/trn_rl_repo", "/root/.axon_site/_ro/trn_rl_repo"):
    if os.path.isdir(_p) and _p not in sys.path:
        sys.path.insert(0, _p)

import ml_dtypes
import numpy as np

import concourse.bass as bass
import concourse.mybir as mybir
import concourse.tile as tile
from concourse import bacc, library_config

BF16 = mybir.dt.bfloat16
F32 = mybir.dt.float32

B, N, DIM, H, HD = 64, 197, 768, 12, 64
NCORES = 8
BL = B // NCORES          # 8 images per core
TOK = BL * N              # 1576 tokens per core
C = 6                     # contraction chunks of 128 (768 = 6*128, no pad row)
CP = C * 128              # 768
NQ = 394                  # qk/out-projection free chunk (4 * 394 = 1576)
PRJ = 384                 # v-projection free chunk (2 * 384 = 768)
N2 = 2 * N                # paired scores free size (keys 0:128 | keys 128:197)
FT = 2 * DIM // 128       # 12 q/k feature tiles (0-5: q, 6-11: k)

MUL = mybir.AluOpType.mult
IDENT = mybir.ActivationFunctionType.Identity

V_DENSE_IMGS = 2          # images v-projected before attention starts


def _ldw_sig(inst):
    """Identity signature of an InstLdweights' stationary operand."""
    try:
        ap = inst.ins[0]
        return (
            ap.memref, ap.offset, tuple(map(tuple, ap.ap)), str(ap.dtype),
            str(inst.perf_mode), str(inst.is_transpose),
            str(inst.tile_position),
        )
    except Exception:
        return None


def _dedupe_ldweights(nc):
    """Drop LDWEIGHTS whose stationary operand is identical to the previous
    LDWEIGHTS on the PE queue (the array keeps its weights across matmuls).

    Runs on the scheduled stream before finalize: at that point semaphore
    waits still live on the matmuls, so a redundant LDWEIGHTS with no
    sync_info can be removed without breaking synchronization. The GEMM
    loops are ordered contraction-outer over paired PSUM banks specifically
    to create these adjacent same-weights pairs.
    """
    removed = 0
    for blk in nc.m.functions[0].blocks:
        last_sig = None
        todel = []
        for idx, inst in enumerate(blk.instructions):
            if isinstance(inst, mybir.InstLdweights):
                sig = _ldw_sig(inst)
                si = inst.sync_info
                clean = si is None or (
                    len(si.on_wait) == 0 and len(si.on_update) == 0
                )
                if sig is not None and sig == last_sig and clean:
                    todel.append(idx)
                    removed += 1
                else:
                    last_sig = sig
        for idx in reversed(todel):
            del blk.instructions[idx]
    return removed


def build_module() -> bass.Bass:
    nc = bacc.Bacc()
    xt_d = nc.declare_dram_parameter("xt", [CP, TOK], BF16, isOutput=False)
    wqk_d = nc.declare_dram_parameter("wqk", [CP, 2 * DIM], BF16, isOutput=False)
    wv_d = nc.declare_dram_parameter("wv", [CP, DIM], BF16, isOutput=False)
    wp_d = nc.declare_dram_parameter("wp", [CP, DIM], BF16, isOutput=False)
    eb_d = nc.declare_dram_parameter("ebias", [128, H // 2, 2 * N2], BF16, isOutput=False)
    qb_d = nc.declare_dram_parameter("qbias", [CP, 1], F32, isOutput=False)
    pb_d = nc.declare_dram_parameter("pbias", [CP, 1], F32, isOutput=False)
    out_d = nc.declare_dram_parameter("out", [DIM, TOK], BF16, isOutput=True)

    with tile.TileContext(nc) as tc:
        with (
            tc.tile_pool(name="persist", bufs=1) as persist,
            tc.tile_pool(name="sb_e", bufs=3) as sb_e,
            tc.tile_pool(name="sb_e2", bufs=5) as sb_e2,
            tc.tile_pool(name="sb_r", bufs=6) as sb_r,
            tc.tile_pool(name="sb_rb", bufs=4) as sb_rb,
            tc.tile_pool(name="sb_out", bufs=6) as sb_out,
        ):
            xt = persist.tile([128, C, TOK], BF16)
            wqk = persist.tile([128, C, 2 * DIM], BF16)
            wv = persist.tile([128, C, DIM], BF16)
            wp = persist.tile([128, C, DIM], BF16)
            eb = persist.tile([128, H // 2, 2, 2, N], BF16)  # exp(bias), pair-major
            qb = persist.tile([128, C, 1], F32)
            pb = persist.tile([128, C, 1], F32)
            # f 0-5: qT, 6-11: kT; +64 zero tail columns let the second
            # scores matmul always run M=128 (keys q0+128 .. q0+256)
            qkT = persist.tile([128, FT, TOK + 64], BF16)
            vst = persist.tile([128, 2 * BL, H, HD + 1], BF16)
            aoT = persist.tile([128, C, TOK], BF16)  # 6 feature chunks
            nc.gpsimd.load_library(library_config.proxy)
            nc.gpsimd.memset(qkT[:, :, TOK:TOK + 64], 0.0)
            # ones columns for the denominator rows, all slots upfront
            nc.gpsimd.memset(vst[:, :, :, HD:HD + 1], 1.0)

            # wave 1 (sync, chunk-granular so early chunks land early):
            # exactly what the v-dense phase needs. Later waves follow on
            # the SAME queue so they cannot steal ring bandwidth from it.
            vtok = V_DENSE_IMGS * N  # 394
            for c in range(C):
                nc.sync.dma_start(
                    xt[:, c, 0:vtok], xt_d[c * 128:(c + 1) * 128, 0:vtok]
                )
                nc.sync.dma_start(
                    wv[:, c, 0:PRJ], wv_d[c * 128:(c + 1) * 128, 0:PRJ]
                )
            for c in range(C):
                nc.sync.dma_start(
                    wv[:, c, PRJ:DIM], wv_d[c * 128:(c + 1) * 128, PRJ:DIM]
                )
            for c in range(C):
                nc.sync.dma_start(
                    xt[:, c, vtok:2 * NQ], xt_d[c * 128:(c + 1) * 128, vtok:2 * NQ]
                )
            # wqk per-feature-tile so the qk0 phase can start on f=0 while
            # later tiles are still in flight
            for f in range(FT):
                nc.sync.dma_start(
                    wqk[:, :, f * 128:(f + 1) * 128],
                    wqk_d[:, f * 128:(f + 1) * 128].rearrange(
                        "(c p) d -> p c d", p=128
                    ),
                )
            nc.sync.dma_start(
                eb[:, 0:2, :, :, :].rearrange("p h i half n -> p h (i half n)"),
                eb_d[:, 0:2, :],
            )
            nc.sync.dma_start(
                eb[:, 2:H // 2, :, :, :].rearrange("p h i half n -> p h (i half n)"),
                eb_d[:, 2:H // 2, :],
            )
            nc.sync.dma_start(
                xt[:, :, 2 * NQ:TOK],
                xt_d[:, 2 * NQ:TOK].rearrange("(c p) t -> p c t", p=128),
            )
            nc.sync.dma_start(wp[:], wp_d[:].rearrange("(c p) d -> p c d", p=128))
            # small tables on the scalar queue (land early, tiny traffic)
            nc.scalar.dma_start(qb[:], qb_d[:].rearrange("(c p) o -> p c o", p=128))
            nc.scalar.dma_start(pb[:], pb_d[:].rearrange("(c p) o -> p c o", p=128))

            def v_group_args(b, n_major=False):
                out = []
                order = (
                    [(n, t) for n in range(2) for t in range(2)]
                    if n_major else
                    [(n, t) for t in range(2) for n in range(2)]
                )
                for n, t in order:
                    m = 128 if t == 0 else N - 128
                    out.append((b * 2 + t, m, b * N + t * 128, n))
                return out

            def v_group_body(pool, bt, m, tok0, n, tag="v"):
                ps = pool.tile([128, PRJ], F32, tag=tag, name=f"v_{bt}_{n}")
                for c in range(C):
                    nc.tensor.matmul(
                        ps[0:m, :],
                        lhsT=xt[:, c, tok0:tok0 + m],
                        rhs=wv[:, c, n * PRJ:(n + 1) * PRJ],
                        start=(c == 0),
                        stop=(c == C - 1),
                    )
                # v copy on DVE: the Act engine carries exp + k/proj copies,
                # DVE has the headroom here
                nc.vector.tensor_copy(
                    vst[0:m, bt, n * 6:(n + 1) * 6, 0:HD],
                    ps[0:m, :].rearrange("p (h d) -> p h d", d=HD),
                )

            # ---- v projection for images 0..V_DENSE_IMGS-1 (covers the
            # input-DMA ramp); the rest is deferred attention filler.
            with tc.tile_pool(name="ps_v", bufs=8, space="PSUM") as ps_v:
                for b in range(V_DENSE_IMGS):
                    for args in v_group_args(b, n_major=True):
                        v_group_body(ps_v, *args)

            def emit_qk_group(pool, f, n, tag="qk"):
                ps = pool.tile([128, NQ], F32, tag=tag, name=f"qk_{f}_{n}")
                for c in range(C):
                    nc.tensor.matmul(
                        ps[:, :],
                        lhsT=wqk[:, c, f * 128:(f + 1) * 128],
                        rhs=xt[:, c, n * NQ:(n + 1) * NQ],
                        start=(c == 0),
                        stop=(c == C - 1),
                    )
                if f < FT // 2:
                    # q tiles: add the (pre-scaled) per-partition q bias
                    # during the PSUM->SBUF copy. Both q and k copies live
                    # on Act (identity/copy share exp's act-table set); the
                    # exps are emitted ahead of them in each pair so the
                    # strict Act FIFO never delays the softmax chain.
                    nc.scalar.activation(
                        qkT[:, f, n * NQ:(n + 1) * NQ], ps[:, :], IDENT,
                        bias=qb[:, f, 0:1],
                    )
                else:
                    nc.scalar.copy(qkT[:, f, n * NQ:(n + 1) * NQ], ps[:, :])

            # ---- q/k projection chunk n=0 (all features): image 0's pairs
            # need it; later chunks feed in as attention filler.
            with tc.tile_pool(name="ps_qk0", bufs=8, space="PSUM") as ps_qk0:
                for f in range(FT):
                    emit_qk_group(ps_qk0, f, 0)

            # ---- attention with interleaved filler
            with (
                tc.tile_pool(name="ps_s", bufs=2, space="PSUM") as ps_s,
                tc.tile_pool(name="ps_o", bufs=3, space="PSUM") as ps_o,
                tc.tile_pool(name="ps_f", bufs=3, space="PSUM") as ps_f,
            ):
                proj_ready = []

                def emit_proj_group(d, t0, w):
                    # transposed projection: outT[d-tile, tokens t0:t0+w]
                    ps = ps_f.tile([128, NQ], F32, tag="f", name=f"pp_{d}_{t0}")
                    for c in range(C):
                        nc.tensor.matmul(
                            ps[:, 0:w],
                            lhsT=wp[:, c, d * 128:(d + 1) * 128],
                            rhs=aoT[:, c, t0:t0 + w],
                            start=(c == 0),
                            stop=(c == C - 1),
                        )
                    ob = sb_out.tile([128, NQ], BF16, tag="ob", name=f"ob_{d}_{t0}")
                    nc.scalar.activation(ob[:, 0:w], ps[:, 0:w], IDENT, bias=pb[:, d, 0:1])
                    nc.sync.dma_start(
                        out_d[d * 128:(d + 1) * 128, t0:t0 + w], ob[:, 0:w]
                    )

                # earliest-deadline-first filler queue: qk chunk n is needed
                # by the first image whose scores read it (n=1 -> image 1,
                # n=2 -> image 3, n=3 -> image 5); v(b) by image b's pairs.
                items = []
                for nn in (1, 2, 3):
                    items += [(2 * nn - 1, ("qk", f, nn)) for f in range(FT)]
                for b in range(V_DENSE_IMGS, BL):
                    for args in v_group_args(b):
                        items.append((b, ("vg", args)))
                items.sort(key=lambda x: x[0])
                edf = items  # list of (deadline_image, item)

                def emit_filler_item(it):
                    if it[0] == "qk":
                        _, f, nn = it
                        emit_qk_group(ps_f, f, nn, tag="f")
                    else:
                        v_group_body(ps_f, *it[1], tag="f")

                def pop_filler():
                    # deadline items first: the deadline-free projection
                    # tiles are the only filler the late images can get, so
                    # save them; once the deadline queue is dry, feed two
                    # projection tiles per pop while the backlog is deep
                    if edf:
                        emit_filler_item(edf.pop(0)[1])
                    elif proj_ready:
                        emit_proj_group(*proj_ready.pop(0))
                        if len(proj_ready) > 4:
                            emit_proj_group(*proj_ready.pop(0))

                def emit_norm(b, hp, os_p):
                    q0 = b * N
                    pair = (2 * hp, 2 * hp + 1)
                    # one denominator copy / reciprocal / broadcast for the
                    # pair (custom-DVE reciprocal misreads PSUM; stage the
                    # denom rows in SBUF first). GpSimd has nothing else to
                    # do in this design, so the broadcast lives there.
                    rc = sb_r.tile([1, 2, N], F32, tag="rc")
                    nc.scalar.copy(rc[0:1, :, :], os_p[64:65, :, :])
                    rr = sb_r.tile([1, 2, N], F32, tag="rr")
                    nc.vector.reciprocal_approx_fast(rr[0:1, :, :], rc[0:1, :, :])
                    rb = sb_rb.tile([64, 2, N], F32)
                    nc.gpsimd.partition_broadcast(rb[0:64, :, :], rr[0:1, :, :])
                    for i, h in enumerate(pair):
                        po, fq = (h % 2) * 64, h // 2
                        nc.vector.scalar_tensor_tensor(
                            out=aoT[po:po + 64, fq, q0:q0 + N],
                            in0=os_p[0:64, i, :], scalar=1.0,
                            in1=rb[0:64, i, :],
                            op0=MUL, op1=MUL,
                        )
                    # projection tiles unlock once their token span is fully
                    # normalized: full n-chunks after odd images 1/3/5; the
                    # final chunk splits per image (6, then 7) so its first
                    # half feeds the drain earlier
                    if hp == H // 2 - 1:
                        if b % 2 == 1 and b < BL - 1:
                            for d in range(C):
                                proj_ready.append((d, (b // 2) * NQ, NQ))
                        elif b >= BL - 2:
                            for d in range(C):
                                proj_ready.append((d, b * N, N))

                pending = []

                def emit_av(b, hp, es2_t):
                    # out.T (64 rows) + softmax denominators (row 64); both
                    # heads of the pair share one PSUM bank
                    os_p = ps_o.tile([128, 2, N], F32, tag="o", name=f"o_{b}_{hp}")
                    for i, h in enumerate((2 * hp, 2 * hp + 1)):
                        nc.tensor.matmul(
                            os_p[0:HD + 1, i, :], lhsT=vst[:, b * 2, h, :],
                            rhs=es2_t[0:128, i, 0, :], start=True, stop=False,
                        )
                        nc.tensor.matmul(
                            os_p[0:HD + 1, i, :],
                            lhsT=vst[0:69, b * 2 + 1, h, :],
                            rhs=es2_t[0:69, i, 1, :], start=False, stop=True,
                        )
                    pending.append((b, hp, os_p))

                # AV matmuls run three pairs BEHIND: pair p's exp+mult chain
                # has ~3 pair periods of slack, so transient engine-queue
                # jams never stall the PE. Within a pair, the scores/exp/
                # mult are emitted FIRST so they head their engine queues;
                # norm drains and filler copies queue behind them.
                av_q = []
                for b in range(BL):
                    q0 = b * N
                    # force any not-yet-emitted prerequisites of this image
                    # (EDF order makes them a queue prefix)
                    while edf and edf[0][0] <= b:
                        emit_filler_item(edf.pop(0)[1])
                    for hp in range(H // 2):
                        pair = (2 * hp, 2 * hp + 1)
                        ss = {}
                        # scoresT = k @ q.T, one PSUM bank per head, two
                        # disjoint column groups (keys 0:128 | 128:256; the
                        # second spills into the next image / zero tail --
                        # rows 69:128 of that half are never consumed) so
                        # every matmul covers all 128 partitions.
                        for h in pair:
                            ss[h] = ps_s.tile([128, 2, N], F32, tag="s", name=f"s_{b}_{h}")
                            po, fq, fk = (h % 2) * 64, h // 2, FT // 2 + h // 2
                            nc.tensor.matmul(
                                ss[h][0:128, 0, :],
                                lhsT=qkT[po:po + 64, fk, q0:q0 + 128],
                                rhs=qkT[po:po + 64, fq, q0:q0 + N],
                                start=True, stop=True,
                            )
                            nc.tensor.matmul(
                                ss[h][0:128, 1, :],
                                lhsT=qkT[po:po + 64, fk, q0 + 128:q0 + 256],
                                rhs=qkT[po:po + 64, fq, q0:q0 + N],
                                start=True, stop=True,
                            )
                        es = sb_e.tile([128, 2, 2, N], BF16, tag="e", name=f"e_{b}_{hp}")
                        for i, h in enumerate(pair):
                            nc.scalar.activation(
                                es[:, i, :, :], ss[h][:, :, :],
                                mybir.ActivationFunctionType.Exp,
                            )
                        # exp(bias) multiply for the pair, one DVE op; STT
                        # form (not tensor_tensor) so the all-SBUF bf16
                        # operands qualify for the DVE 4x perf mode
                        es2 = sb_e2.tile([128, 2, 2, N], BF16, tag="e2", name=f"e2_{b}_{hp}")
                        nc.vector.scalar_tensor_tensor(
                            out=es2[:, :, :, :], in0=es[:, :, :, :], scalar=1.0,
                            in1=eb[:, hp, :, :, :], op0=MUL, op1=MUL,
                        )
                        # deferred normalization + filler AFTER the chain
                        # ops so they queue behind exp/mult on Act/DVE; the
                        # last image drains eagerly so its projection tiles
                        # unlock in time to fill the final pairs
                        keep = 0 if b == BL - 1 else 1
                        while len(pending) > keep:
                            emit_norm(*pending.pop(0))
                        pop_filler()
                        av_q.append((b, hp, es2))
                        if len(av_q) > 3:
                            emit_av(*av_q.pop(0))

                # drain: interleave unlocked projection tiles between the
                # final AVs/norms so their chains are covered by PE work
                while av_q:
                    emit_av(*av_q.pop(0))
                    while len(pending) > 1:
                        emit_norm(*pending.pop(0))
                    if proj_ready:
                        emit_proj_group(*proj_ready.pop(0))
                while pending:
                    emit_norm(*pending.pop(0))
                while edf:
                    emit_filler_item(edf.pop(0)[1])
                while proj_ready:
                    emit_proj_group(*proj_ready.pop(0))

    n_removed = _dedupe_ldweights(nc)
    if os.environ.get("KERNEL_DEBUG"):
        print(f"deduped {n_removed} LDWEIGHTS")
    nc.finalize()
    return nc


def prep_shared(w_qkv, q_bias, v_bias, rel_table, w_proj, b_proj, rel_index):
    """Host-side weight/bias layouts shared by all cores (bf16)."""
    bf = ml_dtypes.bfloat16
    scale = HD ** -0.5

    wqk = np.empty((CP, 2 * DIM), np.float32)
    wqk[:, 0:DIM] = w_qkv[0:DIM].T * scale
    wqk[:, DIM:2 * DIM] = w_qkv[DIM:2 * DIM].T

    wv = np.ascontiguousarray(w_qkv[2 * DIM:3 * DIM].T)
    wp = np.ascontiguousarray(w_proj.T)
    # softmax weights sum to 1, so the V bias adds a constant v_bias per
    # query; its projected image joins the output-projection bias, applied
    # per-partition in the transposed projection's PSUM->SBUF copy
    pbias = (b_proj + w_proj @ v_bias).astype(np.float32)

    # exp(bias)[q, k, h] -> key-major PAIR layout
    # eb[key%128, hp, (i, half), q] with heads (2hp, 2hp+1) interleaved on
    # the i dim; multiplied into the exp'd scores on DVE
    bmat = np.exp(rel_table[rel_index])     # [197(q), 197(k), 12]
    ebt = np.ones((128, H // 2, 2, 2, N), np.float32)
    bt_ = bmat.transpose(1, 2, 0)           # [k, h, q]
    ebt[:, :, :, 0, :] = bt_[0:128].reshape(128, H // 2, 2, N)
    ebt[0:69, :, :, 1, :] = bt_[128:N].reshape(69, H // 2, 2, N)

    return {
        "wqk": wqk.astype(bf),
        "wv": wv.astype(bf),
        "wp": wp.astype(bf),
        "ebias": ebt.reshape(128, H // 2, 2 * N2).astype(bf),
        "qbias": (q_bias * scale).astype(np.float32).reshape(CP, 1),
        "pbias": pbias.reshape(CP, 1),
    }


def prep_core_x(x, core):
    bf = ml_dtypes.bfloat16
    xs = x[core * BL:(core + 1) * BL].reshape(TOK, DIM)
    return np.ascontiguousarray(xs.T).astype(bf)


_built = None


def kernel(**inputs) -> np.ndarray:
    global _built
    from concourse.bass_utils import run_bass_kernel_spmd

    x = np.asarray(inputs["x"], np.float32)
    shared = prep_shared(
        np.asarray(inputs["w_qkv"], np.float32),
        np.asarray(inputs["q_bias"], np.float32),
        np.asarray(inputs["v_bias"], np.float32),
        np.asarray(inputs["rel_table"], np.float32),
        np.asarray(inputs["w_proj"], np.float32),
        np.asarray(inputs["b_proj"], np.float32),
        np.asarray(inputs["rel_index"], np.int32),
    )
    in_maps = [dict(shared, xt=prep_core_x(x, i)) for i in range(NCORES)]

    if _built is None:
        _built = (None, build_module())
    res = run_bass_kernel_spmd(_built[1], in_maps, core_ids=list(range(NCORES)))
    out = np.concatenate(
        [
            np.asarray(res.results[i]["out"]).T.reshape(BL, N, DIM)
            for i in range(NCORES)
        ],
        axis=0,
    )
    return out.astype(np.float32)


if __name__ == "__main__":
    nc = build_module()
    print("build OK:", len(nc.m.functions[0].blocks[0].instructions), "instructions?")
